# revision 9
# baseline (speedup 1.0000x reference)
"""Block-causal attention (BlockDiffusionDecoder) on 8 TRN2 NeuronCores.

Reference computes, per (b, h):
    S = (Q K^T) / 8, masked so query block i (64 rows) attends key blocks <= i,
    O = softmax(S) V,   shapes [2, 16, 2048, 64] f32.

Sharding: batch*heads (32) split across 8 cores, 4 heads per core, no comm.

Per-core algorithm (all matmuls bf16, fp32 accumulate):
  - Q,K are staged to DRAM bf16 [2048, 128] (two heads side by side) and
    transposed via the DMA xbar into SBUF [128, 2048]: partitions 0:64 hold
    head-even Q^T/K^T, 64:128 head-odd. The two heads of a pair are
    processed interleaved: their QK^T matmuls are 64-contraction row-tiled
    ops on disjoint partition halves, so the PE runs them concurrently.
  - Scores are computed transposed (S^T tile [128 k, 256 q]): stationary
    K^T_j [64, 128], moving Q^T [64, 256]. Both heads' scores for a
    3-k-tile group share one PSUM tile [128, 1536] -> one exp call.
  - exp via ScalarE from PSUM, no max-subtraction (|scores/8| <= ~6 so
    fp32/bf16 exp is safe), output P^T in SBUF bf16.
  - Block mask applied by zeroing P^T sub-blocks (DVE memset) or by skipping
    the fully-masked half-block in the PV read.
  - PV: stationary V'_j [128 k, 65] (V plus a ones column -> row sums land
    in output row 64), moving P^T, accumulating O^T [65, 256] in PSUM.
  - O^T -> O via DMA xbar transpose (bf16), then normalize rows by the
    reciprocal of the softmax sum on VectorE and DMA out as f32.
  - Input DMA work is linked into a need-ordered dependency chain so the
    scheduler can't starve a critical transfer behind prefetches.
"""

import numpy as np

B, H, S, D = 2, 16, 2048, 64
N_CORES = 8
HPC = (B * H) // N_CORES  # heads per core = 4
NP = HPC // 2  # head pairs per core = 2
QP = S // 256  # q-pairs (two 128-row q-tiles per step) = 8
GW = 1536  # PSUM score-group width: 3 k-tiles x 256 q, both heads

_CACHE = {}


def _build():
    import concourse.bass as bass
    import concourse.mybir as mybir
    from concourse import bacc
    from concourse.bass import ts
    from concourse.tile import TileContext
    from concourse.tile_rust import add_dep_helper

    f32 = mybir.dt.float32
    bf16 = mybir.dt.bfloat16

    nc = bacc.Bacc("TRN2", target_bir_lowering=False, debug=False,
                   num_devices=N_CORES)
    q = nc.declare_dram_parameter("q", [HPC, S, D], f32, isOutput=False)
    k = nc.declare_dram_parameter("k", [HPC, S, D], f32, isOutput=False)
    v = nc.declare_dram_parameter("v", [HPC, S, D], f32, isOutput=False)
    out = nc.declare_dram_parameter("out", [HPC, S, D], f32, isOutput=True)

    def off(j, e):  # free offset of k-tile j (head-half e) inside pT
        return (j // 3) * GW + e * 768 + (j % 3) * 256

    chain_prev = [None]

    def chained(bi):
        if chain_prev[0] is not None:
            add_dep_helper(bi.ins, chain_prev[0].ins, sync=False,
                           reason="dma need-order")
        chain_prev[0] = bi
        return bi

    with TileContext(nc) as tc:
        with (
            tc.tile_pool(name="dram_stage", bufs=2, space="DRAM") as dpool,
            tc.tile_pool(name="qkT", bufs=2) as qkT_pool,
            tc.tile_pool(name="vsb", bufs=4) as v_pool,
            tc.tile_pool(name="pT", bufs=3) as pT_pool,
            tc.tile_pool(name="osb", bufs=4) as o_pool,
            tc.tile_pool(name="ps", bufs=2, space="PSUM") as ps_pool,
            tc.tile_pool(name="oT", bufs=2, space="PSUM") as oT_pool,
        ):
            qT = {}
            kT = {}
            v_sb = {}
            oT_sb = {}

            def emit_pair_stage(p, halves, between=None):
                sq = dpool.tile([S, 128], bf16, name=f"sq{p}", tag="sq")
                sk = dpool.tile([S, 128], bf16, name=f"sk{p}", tag="sk")
                qT_t = qkT_pool.tile([128, S], bf16, name=f"qT{p}", tag="qT")
                kT_t = qkT_pool.tile([128, S], bf16, name=f"kT{p}", tag="kT")
                hs = S // halves
                for a in range(halves):
                    if a > 0 and between is not None:
                        between()
                    sl = slice(a * hs, (a + 1) * hs)
                    chained(nc.gpsimd.dma_start(
                        out=sq[sl, :].rearrange("s (e d) -> s e d", e=2),
                        in_=q[2 * p:2 * p + 2, sl].rearrange("e s d -> s e d"),
                    ))
                    chained(nc.gpsimd.dma_start(
                        out=sk[sl, :].rearrange("s (e d) -> s e d", e=2),
                        in_=k[2 * p:2 * p + 2, sl].rearrange("e s d -> s e d"),
                    ))
                    chained(nc.sync.dma_start_transpose(
                        out=qT_t[:, sl], in_=sq[sl, :]))
                    chained(nc.sync.dma_start_transpose(
                        out=kT_t[:, sl], in_=sk[sl, :]))
                qT[p], kT[p] = qT_t, kT_t

            def emit_head_pre(h):
                vs = v_pool.tile([128, 16, 65], bf16, name=f"v{h}", tag="v")
                chained(nc.gpsimd.dma_start(
                    out=vs[:, :, 0:64],
                    in_=v[h].rearrange("(n p) d -> p n d", p=128),
                ))
                nc.vector.memset(vs[:, :, 64], 1.0)
                v_sb[h] = vs
                oT_sb[h] = o_pool.tile([128, S], bf16, name=f"o{h}", tag="osb")

            def emit_scores(p, t):
                """Both heads of pair p, q-pair t: matmuls + exp + mask."""
                jmax = 2 * t + 1
                ngroups = (jmax + 3) // 3
                pT = pT_pool.tile([128, 6 * GW], bf16, name=f"pT_{p}_{t}",
                                  tag="pT")
                for g in range(ngroups):
                    jn = min(3, jmax + 1 - 3 * g)
                    ps = ps_pool.tile([128, GW], mybir.dt.float32,
                                      name=f"ps_{p}_{t}_{g}", tag="ps")
                    for jj in range(jn):
                        j = 3 * g + jj
                        for e in range(2):  # interleave heads: row-tile pair
                            nc.tensor.matmul(
                                ps[:, e * 768 + jj * 256:e * 768 + (jj + 1) * 256],
                                kT[p][64 * e:64 * e + 64, ts(j, 128)],
                                qT[p][64 * e:64 * e + 64, t * 256:(t + 1) * 256],
                                start=True, stop=True,
                            )
                    if jn == 3:
                        nc.scalar.activation(
                            pT[:, g * GW:(g + 1) * GW], ps[:],
                            mybir.ActivationFunctionType.Exp, scale=0.125)
                    else:
                        w = jn * 256
                        nc.scalar.activation(
                            pT[:, g * GW:(g + 1) * GW].rearrange(
                                "p (e r) -> p e r", e=2)[:, :, :w],
                            ps[:].rearrange("p (e r) -> p e r", e=2)[:, :, :w],
                            mybir.ActivationFunctionType.Exp, scale=0.125)
                for e in range(2):
                    a = off(2 * t, e)
                    nc.vector.memset(pT[64:128, a:a + 64], 0.0)
                    b = off(2 * t + 1, e)
                    nc.vector.memset(pT[64:128, b + 128:b + 192], 0.0)
                return pT

            def emit_pv(p, t, pT):
                jmax = 2 * t + 1
                for e in range(2):
                    h = 2 * p + e
                    oT = oT_pool.tile([65, 256], mybir.dt.float32,
                                      name=f"oT_{h}_{t}", tag="oT")
                    for j in range(jmax):
                        nc.tensor.matmul(
                            oT[:], v_sb[h][:, j, :],
                            pT[:, off(j, e):off(j, e) + 256],
                            start=(j == 0), stop=False,
                        )
                    # k-tile jmax: q-tile 2t is fully below it -> only the
                    # right 128 columns (q-tile 2t+1) see it.
                    bo = off(jmax, e)
                    nc.tensor.matmul(
                        oT[:, 128:256], v_sb[h][:, jmax, :],
                        pT[:, bo + 128:bo + 256],
                        start=False, stop=True,
                    )
                    nc.vector.tensor_copy(
                        oT_sb[h][0:65, t * 256:(t + 1) * 256], oT[:])

            def emit_head_post(h):
                ot = o_pool.tile([128, 16, 80], bf16, name=f"ot{h}", tag="ot")
                nc.sync.dma_start_transpose(out=ot[:], in_=oT_sb[h][0:80, :])
                rec = o_pool.tile([128, 16], mybir.dt.float32,
                                  name=f"rec{h}", tag="rec")
                nc.vector.reciprocal(rec[:], ot[:, :, 64])
                of = o_pool.tile([128, 16, 64], mybir.dt.float32,
                                 name=f"of{h}", tag="of")
                for n in range(16):
                    nc.vector.tensor_scalar_mul(
                        of[:, n, :], ot[:, n, 0:64], rec[:, n:n + 1])
                nc.sync.dma_start(
                    out=out[h].rearrange("(n p) d -> p n d", p=128), in_=of[:])

            # ---- software-pipelined issue order ----
            emit_pair_stage(0, halves=1)
            emit_head_pre(0)
            emit_head_pre(1)
            items = [(p, t) for p in range(NP) for t in range(QP)]
            pending = None
            for p, t in items:
                pT = emit_scores(p, t)
                if p == 0 and t == 3 and NP > 1:
                    emit_pair_stage(1, halves=1)
                if p == 0 and t == 5 and NP > 1:
                    emit_head_pre(2)
                    emit_head_pre(3)
                if pending is not None:
                    pp, pt, ppT = pending
                    emit_pv(pp, pt, ppT)
                    if pt == QP - 1:
                        emit_head_post(2 * pp)
                        emit_head_post(2 * pp + 1)
                pending = (p, t, pT)
            pp, pt, ppT = pending
            emit_pv(pp, pt, ppT)
            emit_head_post(2 * pp)
            emit_head_post(2 * pp + 1)

    nc.compile()
    return nc


def _get_nc():
    if "nc" not in _CACHE:
        _CACHE["nc"] = _build()
    return _CACHE["nc"]


def kernel(q, k, v):
    from concourse.bass_utils import run_bass_kernel_spmd

    nc = _get_nc()
    qf = np.ascontiguousarray(q, dtype=np.float32).reshape(B * H, S, D)
    kf = np.ascontiguousarray(k, dtype=np.float32).reshape(B * H, S, D)
    vf = np.ascontiguousarray(v, dtype=np.float32).reshape(B * H, S, D)
    in_maps = [
        {
            "q": qf[c * HPC:(c + 1) * HPC],
            "k": kf[c * HPC:(c + 1) * HPC],
            "v": vf[c * HPC:(c + 1) * HPC],
        }
        for c in range(N_CORES)
    ]
    res = run_bass_kernel_spmd(nc, in_maps, core_ids=list(range(N_CORES)))
    full = np.concatenate([res.results[c]["out"] for c in range(N_CORES)], axis=0)
    return full.reshape(B, H, S, D).astype(np.float32)


# revision 22
# speedup vs baseline: 10280.5509x; 10280.5509x over previous
"""Block-causal attention (BlockDiffusionDecoder) on 8 TRN2 NeuronCores.

Reference computes, per (b, h):
    S = (Q K^T) / 8, masked so query block i (64 rows) attends key blocks <= i,
    O = softmax(S) V,   shapes [2, 16, 2048, 64] f32.

Sharding: batch*heads (32) split across 8 cores, 4 heads per core, no comm.

Per-core algorithm (all matmuls bf16, fp32 accumulate):
  - Q,K are staged to DRAM bf16 [2048, 128] (two heads side by side) and
    transposed via the DMA xbar into SBUF [128, 2048]: partitions 0:64 hold
    head-even Q^T/K^T, 64:128 head-odd. The two heads of a pair are
    processed interleaved: their QK^T matmuls are 64-contraction row-tiled
    ops on disjoint partition halves, so the PE runs them concurrently.
  - Scores are computed transposed (S^T tile [128 k, 256 q]): stationary
    K^T_j [64, 128], moving Q^T [64, 256]. Both heads' scores for a
    3-k-tile group share one PSUM tile [128, 1536] -> one exp call.
  - softmax exp without max-subtraction (|scores/8| <= ~6 so fp32/bf16 exp
    is safe). Most groups go through ScalarE's exact exp; the first k-group
    of each q step (t >= 1) is offloaded to VectorE as a one-instruction
    Schraudolph exp2 (scores*A+B computed f32, converted to int32, whose
    high half IS the bf16 result; ~2% element error on ~29% of weights),
    keeping that work off the ScalarE critical path.
  - Block mask applied by zeroing P^T sub-blocks (DVE memset) or by skipping
    the fully-masked half-block in the PV read.
  - PV: stationary V'_j [128 k, 65] (V plus a ones column -> row sums land
    in output row 64), moving P^T, accumulating O^T [65, 256] in PSUM.
  - O^T -> O via DMA xbar transpose (bf16) in 1024-column halves spread
    through the schedule, then normalize rows by the reciprocal of the
    softmax sum on VectorE and DMA out as f32.
  - Input DMA work is linked into a need-ordered dependency chain so the
    scheduler can't starve a critical transfer behind prefetches.
"""

import numpy as np

B, H, S, D = 2, 16, 2048, 64
N_CORES = 8
HPC = (B * H) // N_CORES  # heads per core = 4
NP = HPC // 2  # head pairs per core = 2
QP = S // 256  # q-pairs (two 128-row q-tiles per step) = 8
GW = 1536  # PSUM score-group width: 3 k-tiles x 256 q, both heads

# Schraudolph exp2 constants: exp(0.125*s) ~= bitcast_f32(i32(s*SCH_A + SCH_B))
SCH_A = 8388608.0 * 0.125 * 1.4426950408889634
SCH_B = 127.0 * 8388608.0 - 334700.0

_CACHE = {}
import os
SCHRAUD = int(os.environ.get("SCHRAUD", "0"))


def _build(reps=1):
    import concourse.bass as bass
    import concourse.mybir as mybir
    from concourse import bacc
    from concourse.bass import ts
    from concourse.tile import TileContext
    from concourse.tile_rust import add_dep_helper

    f32 = mybir.dt.float32
    bf16 = mybir.dt.bfloat16

    nc = bacc.Bacc("TRN2", target_bir_lowering=False, debug=False,
                   num_devices=N_CORES)
    q = nc.declare_dram_parameter("q", [HPC, S, D], f32, isOutput=False)
    k = nc.declare_dram_parameter("k", [HPC, S, D], f32, isOutput=False)
    v = nc.declare_dram_parameter("v", [HPC, S, D], f32, isOutput=False)
    out = nc.declare_dram_parameter("out", [HPC, S, D], f32, isOutput=True)

    def off(j, e):  # free offset of k-tile j (head-half e) inside pT
        return (j // 3) * GW + e * 768 + (j % 3) * 256

    chain_prev = [None]

    def chained(bi):
        if chain_prev[0] is not None:
            add_dep_helper(bi.ins, chain_prev[0].ins, sync=False,
                           reason="dma need-order")
        chain_prev[0] = bi
        return bi

    with TileContext(nc) as tc:
        with (
            tc.tile_pool(name="dram_stage", bufs=2, space="DRAM") as dpool,
            tc.tile_pool(name="qkT", bufs=2) as qkT_pool,
            tc.tile_pool(name="vsb", bufs=4) as v_pool,
            tc.tile_pool(name="pT", bufs=3) as pT_pool,
            tc.tile_pool(name="osb", bufs=4) as o_pool,
            tc.tile_pool(name="ps", bufs=2, space="PSUM") as ps_pool,
            tc.tile_pool(name="oT", bufs=2, space="PSUM") as oT_pool,
        ):
            qT = {}
            kT = {}
            v_sb = {}
            oT_sb = {}

            def emit_pair_stage(p, split_first=False):
                sq = dpool.tile([S, 128], bf16, name=f"sq{p}", tag="sq")
                sk = dpool.tile([S, 128], bf16, name=f"sk{p}", tag="sk")
                qT_t = qkT_pool.tile([128, S], bf16, name=f"qT{p}", tag="qT")
                kT_t = qkT_pool.tile([128, S], bf16, name=f"kT{p}", tag="kT")
                chained(nc.gpsimd.dma_start(
                    out=sq[:].rearrange("s (e d) -> s e d", e=2),
                    in_=q[2 * p:2 * p + 2].rearrange("e s d -> s e d"),
                ))
                chained(nc.gpsimd.dma_start(
                    out=sk[:].rearrange("s (e d) -> s e d", e=2),
                    in_=k[2 * p:2 * p + 2].rearrange("e s d -> s e d"),
                ))
                if split_first:
                    chained(nc.sync.dma_start_transpose(
                        out=qT_t[:, 0:256], in_=sq[0:256, :]))
                    chained(nc.sync.dma_start_transpose(
                        out=kT_t[:, 0:256], in_=sk[0:256, :]))
                    chained(nc.sync.dma_start_transpose(
                        out=qT_t[:, 256:S], in_=sq[256:S, :]))
                    chained(nc.sync.dma_start_transpose(
                        out=kT_t[:, 256:S], in_=sk[256:S, :]))
                else:
                    chained(nc.sync.dma_start_transpose(out=qT_t[:], in_=sq[:]))
                    chained(nc.sync.dma_start_transpose(out=kT_t[:], in_=sk[:]))
                qT[p], kT[p] = qT_t, kT_t

            def emit_head_pre(h):
                vs = v_pool.tile([128, 16, 65], bf16, name=f"v{h}", tag="v")
                chained(nc.gpsimd.dma_start(
                    out=vs[:, :, 0:64],
                    in_=v[h].rearrange("(n p) d -> p n d", p=128),
                ))
                nc.vector.memset(vs[:, :, 64], 1.0)
                v_sb[h] = vs
                oT_sb[h] = o_pool.tile([128, S], bf16, name=f"o{h}", tag="osb")

            def emit_scores(p, t):
                """Both heads of pair p, q-pair t. Returns a pread(j, e)
                accessor over the P^T storage (pT tile + optional int view)."""
                jmax = 2 * t + 1
                ngroups = (jmax + 3) // 3
                pT = pT_pool.tile([128, 6 * GW], bf16, name=f"pT_{p}_{t}",
                                  tag="pT")
                zview = None
                for g in range(ngroups):
                    jn = min(3, jmax + 1 - 3 * g)
                    ps = ps_pool.tile([128, GW], mybir.dt.float32,
                                      name=f"ps_{p}_{t}_{g}", tag="ps")
                    for jj in range(jn):
                        j = 3 * g + jj
                        half = 128 if j == jmax else 0
                        for e in range(2):  # interleave heads: row-tile pair
                            nc.tensor.matmul(
                                ps[:, e * 768 + jj * 256 + half:
                                   e * 768 + (jj + 1) * 256],
                                kT[p][64 * e:64 * e + 64, ts(j, 128)],
                                qT[p][64 * e:64 * e + 64,
                                      t * 256 + half:(t + 1) * 256],
                                start=True, stop=True,
                            )
                    if g == 0 and t >= 1 and SCHRAUD:
                        # Schraudolph fast exp on VectorE: one tensor_scalar
                        # with int32 output; its high bytes are the bf16 P^T.
                        zint = pT_pool.tile([128, GW], mybir.dt.int32,
                                            name=f"z_{p}_{t}", tag="zint")
                        nc.vector.tensor_scalar(
                            zint[:], ps[:], SCH_A, SCH_B,
                            mybir.AluOpType.mult, mybir.AluOpType.add)
                        zview = zint[:].bitcast(bf16).rearrange(
                            "q (n two) -> q n two", two=2)
                    elif jn == 3:
                        nc.scalar.activation(
                            pT[:, g * GW:(g + 1) * GW], ps[:],
                            mybir.ActivationFunctionType.Exp, scale=0.125)
                    else:
                        w = jn * 256
                        nc.scalar.activation(
                            pT[:, g * GW:(g + 1) * GW].rearrange(
                                "q (e r) -> q e r", e=2)[:, :, :w],
                            ps[:].rearrange("q (e r) -> q e r", e=2)[:, :, :w],
                            mybir.ActivationFunctionType.Exp, scale=0.125)

                def pread(j, e, c0=0, c1=256, r0=0):
                    base = e * 768 + (j % 3) * 256
                    if j < 3 and zview is not None:
                        return zview[r0:128, base + c0:base + c1, 1:2]
                    o = off(j, e)
                    return pT[r0:128, o + c0:o + c1]

                for e in range(2):
                    nc.vector.memset(pread(2 * t, e, 0, 64, 64), 0.0)
                    nc.vector.memset(pread(2 * t + 1, e, 128, 192, 64), 0.0)
                return pread

            def emit_pv(p, t, pread):
                jmax = 2 * t + 1
                for e in range(2):
                    h = 2 * p + e
                    oT = oT_pool.tile([65, 256], mybir.dt.float32,
                                      name=f"oT_{h}_{t}", tag="oT")
                    for j in range(jmax):
                        nc.tensor.matmul(
                            oT[:], v_sb[h][:, j, :], pread(j, e),
                            start=(j == 0), stop=False,
                        )
                    # k-tile jmax: q-tile 2t is fully below it -> only the
                    # right 128 columns (q-tile 2t+1) see it.
                    nc.tensor.matmul(
                        oT[:, 128:256], v_sb[h][:, jmax, :],
                        pread(jmax, e, 128, 256),
                        start=False, stop=True,
                    )
                    nc.vector.tensor_copy(
                        oT_sb[h][0:65, t * 256:(t + 1) * 256], oT[:])

            def emit_head_post(h):
                ot = o_pool.tile([128, 16, 80], bf16, name=f"ot{h}", tag="ot")
                nc.sync.dma_start_transpose(out=ot[:], in_=oT_sb[h][0:80, :])
                rec = o_pool.tile([128, 16], mybir.dt.float32,
                                  name=f"rec{h}", tag="rec")
                nc.vector.reciprocal(rec[:], ot[:, :, 64])
                of = o_pool.tile([128, 16, 64], mybir.dt.float32,
                                 name=f"of{h}", tag="of")
                for n in range(16):
                    nc.vector.tensor_scalar_mul(
                        of[:, n, :], ot[:, n, 0:64], rec[:, n:n + 1])
                nc.sync.dma_start(
                    out=out[h].rearrange("(n p) d -> p n d", p=128), in_=of[:])

            # ---- software-pipelined issue order ----
            pending = None
            for rep in range(reps):
                emit_pair_stage(0)
                emit_head_pre(0)
                emit_head_pre(1)
                items = [(p, t) for p in range(NP) for t in range(QP)]
                for p, t in items:
                    pread = emit_scores(p, t)
                    if p == 0 and t == 3 and NP > 1:
                        emit_pair_stage(1)
                    if p == 0 and t == 5 and NP > 1:
                        emit_head_pre(2)
                        emit_head_pre(3)
                    if pending is not None:
                        pp, pt, ppread = pending
                        emit_pv(pp, pt, ppread)
                        if pt == QP - 1:
                            emit_head_post(2 * pp)
                            emit_head_post(2 * pp + 1)
                    pending = (p, t, pread)
            pp, pt, ppread = pending
            emit_pv(pp, pt, ppread)
            emit_head_post(2 * pp)
            emit_head_post(2 * pp + 1)

    nc.compile()
    return nc


def _get_nc():
    if "nc" not in _CACHE:
        _CACHE["nc"] = _build()
    return _CACHE["nc"]


def kernel(q, k, v):
    from concourse.bass_utils import run_bass_kernel_spmd

    nc = _get_nc()
    qf = np.ascontiguousarray(q, dtype=np.float32).reshape(B * H, S, D)
    kf = np.ascontiguousarray(k, dtype=np.float32).reshape(B * H, S, D)
    vf = np.ascontiguousarray(v, dtype=np.float32).reshape(B * H, S, D)
    in_maps = [
        {
            "q": qf[c * HPC:(c + 1) * HPC],
            "k": kf[c * HPC:(c + 1) * HPC],
            "v": vf[c * HPC:(c + 1) * HPC],
        }
        for c in range(N_CORES)
    ]
    res = run_bass_kernel_spmd(nc, in_maps, core_ids=list(range(N_CORES)))
    full = np.concatenate([res.results[c]["out"] for c in range(N_CORES)], axis=0)
    return full.reshape(B, H, S, D).astype(np.float32)


# revision 24
# speedup vs baseline: 15269.4937x; 1.4853x over previous
"""Block-causal attention (BlockDiffusionDecoder) on 8 TRN2 NeuronCores.

Reference computes, per (b, h):
    S = (Q K^T) / 8, masked so query block i (64 rows) attends key blocks <= i,
    O = softmax(S) V,   shapes [2, 16, 2048, 64] f32.

Sharding: batch*heads (32) split across 8 cores, 4 heads per core, no comm.

Per-core algorithm (all matmuls bf16, fp32 accumulate):
  - Q,K are staged to DRAM bf16 [2048, 128] (two heads side by side) and
    transposed via the DMA xbar into SBUF [128, 2048]: partitions 0:64 hold
    head-even Q^T/K^T, 64:128 head-odd. The two heads of a pair are
    processed interleaved: their QK^T matmuls are 64-contraction row-tiled
    ops on disjoint partition halves, so the PE runs them concurrently.
  - Scores are computed transposed (S^T tile [128 k, 256 q]): stationary
    K^T_j [64, 128], moving Q^T [64, 256]. Both heads' scores for a
    3-k-tile group share one PSUM tile [128, 1536] -> one exp call.
  - softmax exp without max-subtraction (|scores/8| <= ~6 so fp32/bf16 exp
    is safe), via ScalarE from PSUM. (A VectorE Schraudolph fast-exp offload
    exists behind SCHRAUD=1 but costs accuracy for little model-time gain.)
  - Block mask applied by zeroing P^T sub-blocks (DVE memset) or by skipping
    the fully-masked half-block in the matmuls / PV reads.
  - PV: stationary V'_j [128 k, 65] (V plus a ones column -> row sums land
    in output row 64), moving P^T, accumulating O^T [65, 256] in PSUM.
  - O^T -> O via DMA xbar transpose (bf16), then normalize rows by the
    reciprocal of the softmax sum on VectorE and DMA out as f32.
  - Input DMA work is linked into a need-ordered dependency chain so the
    scheduler can't starve a critical transfer behind prefetches.
"""

import numpy as np

B, H, S, D = 2, 16, 2048, 64
N_CORES = 8
HPC = (B * H) // N_CORES  # heads per core = 4
NP = HPC // 2  # head pairs per core = 2
QP = S // 256  # q-pairs (two 128-row q-tiles per step) = 8
GW = 1536  # PSUM score-group width: 3 k-tiles x 256 q, both heads

# Schraudolph exp2 constants: exp(0.125*s) ~= bitcast_f32(i32(s*SCH_A + SCH_B))
SCH_A = 8388608.0 * 0.125 * 1.4426950408889634
SCH_B = 127.0 * 8388608.0 - 334700.0

_CACHE = {}

# Schraudolph DVE-offload of the first k-group's exp: saves ~4 us model /
# ~20 us ScalarE busy but triples the output error (1.6e-2 vs 6.5e-3
# max-rel against the 2e-2 gate). Kept available but off.
SCHRAUD = 0


def _build(reps=1):
    import concourse.bass as bass
    import concourse.mybir as mybir
    from concourse import bacc
    from concourse.bass import ts
    from concourse.tile import TileContext
    from concourse.tile_rust import add_dep_helper

    f32 = mybir.dt.float32
    bf16 = mybir.dt.bfloat16

    nc = bacc.Bacc("TRN2", target_bir_lowering=False, debug=False,
                   num_devices=N_CORES)
    q = nc.declare_dram_parameter("q", [HPC, S, D], f32, isOutput=False)
    k = nc.declare_dram_parameter("k", [HPC, S, D], f32, isOutput=False)
    v = nc.declare_dram_parameter("v", [HPC, S, D], f32, isOutput=False)
    out = nc.declare_dram_parameter("out", [HPC, S, D], f32, isOutput=True)

    def off(j, e):  # free offset of k-tile j (head-half e) inside pT
        return (j // 3) * GW + e * 768 + (j % 3) * 256

    chain_prev = [None]

    def chained(bi):
        if chain_prev[0] is not None:
            add_dep_helper(bi.ins, chain_prev[0].ins, sync=False,
                           reason="dma need-order")
        chain_prev[0] = bi
        return bi

    with TileContext(nc) as tc:
        with (
            tc.tile_pool(name="dram_stage", bufs=2, space="DRAM") as dpool,
            tc.tile_pool(name="qkT", bufs=2) as qkT_pool,
            tc.tile_pool(name="vsb", bufs=4) as v_pool,
            tc.tile_pool(name="pT", bufs=3) as pT_pool,
            tc.tile_pool(name="osb", bufs=4) as o_pool,
            tc.tile_pool(name="ps", bufs=2, space="PSUM") as ps_pool,
            tc.tile_pool(name="oT", bufs=2, space="PSUM") as oT_pool,
        ):
            qT = {}
            kT = {}
            v_sb = {}
            oT_sb = {}

            def emit_pair_stage(p, split_first=False):
                sq = dpool.tile([S, 128], bf16, name=f"sq{p}", tag="sq")
                sk = dpool.tile([S, 128], bf16, name=f"sk{p}", tag="sk")
                qT_t = qkT_pool.tile([128, S], bf16, name=f"qT{p}", tag="qT")
                kT_t = qkT_pool.tile([128, S], bf16, name=f"kT{p}", tag="kT")
                chained(nc.gpsimd.dma_start(
                    out=sq[:].rearrange("s (e d) -> s e d", e=2),
                    in_=q[2 * p:2 * p + 2].rearrange("e s d -> s e d"),
                ))
                chained(nc.gpsimd.dma_start(
                    out=sk[:].rearrange("s (e d) -> s e d", e=2),
                    in_=k[2 * p:2 * p + 2].rearrange("e s d -> s e d"),
                ))
                if split_first:
                    chained(nc.sync.dma_start_transpose(
                        out=qT_t[:, 0:256], in_=sq[0:256, :]))
                    chained(nc.sync.dma_start_transpose(
                        out=kT_t[:, 0:256], in_=sk[0:256, :]))
                    chained(nc.sync.dma_start_transpose(
                        out=qT_t[:, 256:S], in_=sq[256:S, :]))
                    chained(nc.sync.dma_start_transpose(
                        out=kT_t[:, 256:S], in_=sk[256:S, :]))
                else:
                    chained(nc.sync.dma_start_transpose(out=qT_t[:], in_=sq[:]))
                    chained(nc.sync.dma_start_transpose(out=kT_t[:], in_=sk[:]))
                qT[p], kT[p] = qT_t, kT_t

            def emit_head_pre(h):
                vs = v_pool.tile([128, 16, 65], bf16, name=f"v{h}", tag="v")
                chained(nc.gpsimd.dma_start(
                    out=vs[:, :, 0:64],
                    in_=v[h].rearrange("(n p) d -> p n d", p=128),
                ))
                nc.vector.memset(vs[:, :, 64], 1.0)
                v_sb[h] = vs
                oT_sb[h] = o_pool.tile([128, S], bf16, name=f"o{h}", tag="osb")

            def emit_scores(p, t):
                """Both heads of pair p, q-pair t. Returns a pread(j, e)
                accessor over the P^T storage (pT tile + optional int view)."""
                jmax = 2 * t + 1
                ngroups = (jmax + 3) // 3
                pT = pT_pool.tile([128, 6 * GW], bf16, name=f"pT_{p}_{t}",
                                  tag="pT")
                zview = None
                for g in range(ngroups):
                    jn = min(3, jmax + 1 - 3 * g)
                    ps = ps_pool.tile([128, GW], mybir.dt.float32,
                                      name=f"ps_{p}_{t}_{g}", tag="ps")
                    for jj in range(jn):
                        j = 3 * g + jj
                        half = 128 if j == jmax else 0
                        for e in range(2):  # interleave heads: row-tile pair
                            nc.tensor.matmul(
                                ps[:, e * 768 + jj * 256 + half:
                                   e * 768 + (jj + 1) * 256],
                                kT[p][64 * e:64 * e + 64, ts(j, 128)],
                                qT[p][64 * e:64 * e + 64,
                                      t * 256 + half:(t + 1) * 256],
                                start=True, stop=True,
                            )
                    if g == 0 and t >= 1 and SCHRAUD:
                        # Schraudolph fast exp on VectorE: one tensor_scalar
                        # with int32 output; its high bytes are the bf16 P^T.
                        zint = pT_pool.tile([128, GW], mybir.dt.int32,
                                            name=f"z_{p}_{t}", tag="zint")
                        nc.vector.tensor_scalar(
                            zint[:], ps[:], SCH_A, SCH_B,
                            mybir.AluOpType.mult, mybir.AluOpType.add)
                        zview = zint[:].bitcast(bf16).rearrange(
                            "q (n two) -> q n two", two=2)
                    elif jn == 3:
                        nc.scalar.activation(
                            pT[:, g * GW:(g + 1) * GW], ps[:],
                            mybir.ActivationFunctionType.Exp, scale=0.125)
                    else:
                        w = jn * 256
                        nc.scalar.activation(
                            pT[:, g * GW:(g + 1) * GW].rearrange(
                                "q (e r) -> q e r", e=2)[:, :, :w],
                            ps[:].rearrange("q (e r) -> q e r", e=2)[:, :, :w],
                            mybir.ActivationFunctionType.Exp, scale=0.125)

                def pread(j, e, c0=0, c1=256, r0=0):
                    base = e * 768 + (j % 3) * 256
                    if j < 3 and zview is not None:
                        return zview[r0:128, base + c0:base + c1, 1:2]
                    o = off(j, e)
                    return pT[r0:128, o + c0:o + c1]

                for e in range(2):
                    nc.vector.memset(pread(2 * t, e, 0, 64, 64), 0.0)
                    nc.vector.memset(pread(2 * t + 1, e, 128, 192, 64), 0.0)
                return pread

            def emit_pv(p, t, pread):
                jmax = 2 * t + 1
                for e in range(2):
                    h = 2 * p + e
                    oT = oT_pool.tile([65, 256], mybir.dt.float32,
                                      name=f"oT_{h}_{t}", tag="oT")
                    for j in range(jmax):
                        nc.tensor.matmul(
                            oT[:], v_sb[h][:, j, :], pread(j, e),
                            start=(j == 0), stop=False,
                        )
                    # k-tile jmax: q-tile 2t is fully below it -> only the
                    # right 128 columns (q-tile 2t+1) see it.
                    nc.tensor.matmul(
                        oT[:, 128:256], v_sb[h][:, jmax, :],
                        pread(jmax, e, 128, 256),
                        start=False, stop=True,
                    )
                    nc.vector.tensor_copy(
                        oT_sb[h][0:65, t * 256:(t + 1) * 256], oT[:])

            def emit_head_post(h):
                ot = o_pool.tile([128, 16, 80], bf16, name=f"ot{h}", tag="ot")
                nc.sync.dma_start_transpose(out=ot[:], in_=oT_sb[h][0:80, :])
                rec = o_pool.tile([128, 16], mybir.dt.float32,
                                  name=f"rec{h}", tag="rec")
                nc.vector.reciprocal(rec[:], ot[:, :, 64])
                of = o_pool.tile([128, 16, 64], mybir.dt.float32,
                                 name=f"of{h}", tag="of")
                for n in range(16):
                    nc.vector.tensor_scalar_mul(
                        of[:, n, :], ot[:, n, 0:64], rec[:, n:n + 1])
                nc.sync.dma_start(
                    out=out[h].rearrange("(n p) d -> p n d", p=128), in_=of[:])

            # ---- software-pipelined issue order ----
            pending = None
            for rep in range(reps):
                emit_pair_stage(0)
                emit_head_pre(0)
                emit_head_pre(1)
                items = [(p, t) for p in range(NP) for t in range(QP)]
                for p, t in items:
                    pread = emit_scores(p, t)
                    if p == 0 and t == 3 and NP > 1:
                        emit_pair_stage(1)
                    if p == 0 and t == 5 and NP > 1:
                        emit_head_pre(2)
                        emit_head_pre(3)
                    if pending is not None:
                        pp, pt, ppread = pending
                        emit_pv(pp, pt, ppread)
                        if pt == QP - 1:
                            emit_head_post(2 * pp)
                            emit_head_post(2 * pp + 1)
                    pending = (p, t, pread)
            pp, pt, ppread = pending
            emit_pv(pp, pt, ppread)
            emit_head_post(2 * pp)
            emit_head_post(2 * pp + 1)

    nc.compile()
    return nc


def _get_nc():
    if "nc" not in _CACHE:
        _CACHE["nc"] = _build()
    return _CACHE["nc"]


def kernel(q, k, v):
    from concourse.bass_utils import run_bass_kernel_spmd

    nc = _get_nc()
    qf = np.ascontiguousarray(q, dtype=np.float32).reshape(B * H, S, D)
    kf = np.ascontiguousarray(k, dtype=np.float32).reshape(B * H, S, D)
    vf = np.ascontiguousarray(v, dtype=np.float32).reshape(B * H, S, D)
    in_maps = [
        {
            "q": qf[c * HPC:(c + 1) * HPC],
            "k": kf[c * HPC:(c + 1) * HPC],
            "v": vf[c * HPC:(c + 1) * HPC],
        }
        for c in range(N_CORES)
    ]
    res = run_bass_kernel_spmd(nc, in_maps, core_ids=list(range(N_CORES)))
    full = np.concatenate([res.results[c]["out"] for c in range(N_CORES)], axis=0)
    return full.reshape(B, H, S, D).astype(np.float32)
